# revision 4
# baseline (speedup 1.0000x reference)
"""Trainium2 kernel: depthwise (channel-multiplier-2) 3x3 conv + wing-swap + add.

Reference computes, for input x (B=32, C=256, H=W=56) and weights w (512,1,3,3):
    y[:, 2i], y[:, 2i+1] = conv3x3(x[:, i], w[2i]), conv3x3(x[:, i], w[2i+1])
    out[:, c] = y[:, 2c] + y[:, 2*swap(c)+1]
where swap() exchanges the two 4-channel wings inside each 8-channel butterfly.
Equivalently:  out[:, c] = conv3x3(x[:, c], w[2c]) + conv3x3(x[:, sc], w[2sc+1]),
sc = swap(c).

Strategy (8 NeuronCores, data-parallel over batch, 4 images/core):
  - channels on SBUF partitions, spatial pixels on the free dim
  - host pre-pads W by 1 col each side (zeros) so every tap reads in-bounds
    and every matmul writes a full, contiguous PSUM window
  - per (image, 128-channel half): 9 per-tap 128x128 block-diagonal matmuls
    (fp16 operands, 1 cycle/column, fp32 PSUM accumulate); the wing swap is
    folded into the per-tap weight matrices host-side (2 nonzeros per
    output-channel column)
  - input rows stream in halo'd chunk tiles per unit (one per 4-row-block
    group) for fast start and DMA/compute overlap; ScalarE evacuates
    PSUM->SBUF
  - within a group the tap loop is OUTER, so the 4 matmuls of one tap share
    their stationary weights back-to-back; a post-compile pass
    (_dedup_ldweights) then drops the redundant InstLdweights (~94 ns each
    on HW), and the surviving per-tap loads hide in the background weight
    buffer behind the preceding tap's matmuls
  - measured ~72.6 us/core on HW (was ~106 us without the ldweights dedup),
    absmax rel err ~5e-4 vs the fp32 reference
"""

import sys
from contextlib import ExitStack

import numpy as np

for _p in ("/opt/trn_rl_repo",):
    if _p not in sys.path:
        sys.path.insert(0, _p)

import concourse.bass as bass
import concourse.tile as tile
from concourse import bacc, mybir
from concourse.bass_utils import run_bass_kernel_spmd


def _dedup_ldweights(m) -> int:
    """Drop InstLdweights whose weights are already resident in the PE array
    (identical to the previous InstLdweights with only InstMatmult between).
    Waits/updates/deps of a dropped load are merged into its InstMatmult.
    Runs before generate_event_semaphores, which legalizes any multi-waits
    this merge produces. Measured ~94 ns saved per dropped load on HW.
    """
    ndrop = 0
    for fn in m.functions:
        for blk in fn.blocks:
            insts = blk.instructions
            last_key = None
            last_kept = None
            pending = []
            remap = {}
            keep = []
            for inst in insts:
                tn = type(inst).__name__
                if inst.engine == mybir.EngineType.PE:
                    if tn == "InstLdweights":
                        key = (
                            str(inst.ins),
                            str(inst.perf_mode),
                            str(inst.is_transpose),
                            str(inst.tile_position),
                        )
                        if key == last_key and last_kept is not None:
                            remap[inst.name] = last_kept
                            pending.append(inst)
                            ndrop += 1
                            continue
                        last_key = key
                        last_kept = inst.name
                    elif tn == "InstMatmult":
                        for drop in pending:
                            si_d = drop.sync_info
                            if si_d is not None and (
                                len(si_d.on_wait) or len(si_d.on_update)
                            ):
                                si_m = inst.sync_info
                                ow = list(si_d.on_wait)
                                ou = list(si_d.on_update)
                                if si_m is not None:
                                    ow = list(si_m.on_wait) + ow
                                    ou = list(si_m.on_update) + ou
                                inst.sync_info = mybir.SyncInfo(
                                    on_wait=ow, on_update=ou
                                )
                            inst.merge_dependencies_from(drop)
                        pending = []
                    else:
                        last_key = None
                        last_kept = None
                keep.append(inst)
            assert not pending, "dropped LDW with no following matmul"
            if len(keep) != len(insts):
                insts.clear()
                insts.extend(keep)
                for inst in insts:
                    inst.remap_dependency_names(remap)
    return ndrop


_orig_move_waits = bacc.Bacc.move_matmul_waits_to_ldweights


def _move_waits_and_dedup(self):
    _orig_move_waits(self)
    if getattr(self, "_dedup_ldw", False):
        self._ndedup = _dedup_ldweights(self.m)


bacc.Bacc.move_matmul_waits_to_ldweights = _move_waits_and_dedup

B, C, H, W = 32, 256, 56, 56
WP = W + 2  # host-padded row width
N_CORES = 8
B_PER = B // N_CORES  # images per core
P = 128               # partitions = channels per half
HALVES = C // P       # 2
RB = 8                # output rows per PSUM block
NRB = H // RB         # 7
NTAPS = 9
BFLY = 8
WING = BFLY // 2

# center tap first: it always writes the full block, so it carries start=True
TAPS = [(0, 0)] + [
    (dh, dw) for dh in (-1, 0, 1) for dw in (-1, 0, 1) if (dh, dw) != (0, 0)
]

_prog_cache = {}


def _swap_local(m: np.ndarray) -> np.ndarray:
    b, r = m // BFLY, m % BFLY
    wng, pos = r // WING, r % WING
    return b * BFLY + (1 - wng) * WING + pos


def _build_weights(w: np.ndarray) -> np.ndarray:
    """Per-tap block-diagonal stationary matrices.

    Returns (P, HALVES*NTAPS*P) f32; wts[k, (h*9+t)*128 + m] is the weight
    from input channel k (partition) to output channel m for tap t of half h.
    """
    w2 = w.reshape(2 * C, NTAPS).astype(np.float32)
    wts = np.zeros((P, HALVES, NTAPS, P), np.float32)
    m = np.arange(P)
    sl = _swap_local(m)
    for h in range(HALVES):
        cg = h * P + m
        sg = h * P + sl
        wts[m, h, :, m] = w2[2 * cg]          # x[c] * w[2c]
        wts[sl, h, :, m] = w2[2 * sg + 1]     # x[sc] * w[2sc+1]
    return np.ascontiguousarray(wts.reshape(P, HALVES * NTAPS * P))


def _build_program(
    loop_iters: int = 1, timing_mode: bool = False, in_dtype: str = "fp16"
) -> bass.Bass:
    # Bacc (not plain Bass): its compile() runs generate_event_semaphores,
    # which splits multi-wait instructions to satisfy the TRN2 1-wait limit
    nc = bacc.Bacc("TRN2", target_bir_lowering=False, debug=False)
    f32 = mybir.dt.float32
    # input dtype trade-off (all run the PE at 1 cycle/column):
    #   f32r: fp32 bits, rel err ~2.2e-4, but 4-byte DMA + slow weight loads
    #   fp16: rel err ~4.1e-4 (11-bit mantissa; |x|<6, |w|<0.5 -> no overflow),
    #         halves input DMA and enables fast (FWL) weight loads
    #   bf16: rel err ~3.6e-3 (8-bit mantissa), same speed as fp16
    in_dt = {
        "f32r": mybir.dt.float32r,
        "fp16": mybir.dt.float16,
        "bf16": mybir.dt.bfloat16,
    }[in_dtype]
    if timing_mode:
        # benchmark-only build: big tensors stay in device DRAM (garbage
        # contents) so wall-time isn't dominated by axon transfers
        x_d = nc.dram_tensor("x_int", [B_PER, C, H, WP], in_dt).ap()
        o_d = nc.dram_tensor("o_int", [B_PER, C, H * W], f32).ap()
        nc.dram_tensor("tiny", [1, 4], f32, kind="ExternalOutput")
    else:
        x_d = nc.dram_tensor("x", [B_PER, C, H, WP], in_dt, kind="ExternalInput").ap()
        o_d = nc.dram_tensor("out", [B_PER, C, H * W], f32, kind="ExternalOutput").ap()
    w_d = nc.dram_tensor("wts", [P, HALVES * NTAPS, P], in_dt, kind="ExternalInput").ap()

    # rb groups (tap loop outer within a group, so consecutive matmuls of a
    # tap share their stationary weights -> _dedup_ldweights drops the
    # redundant loads) and their input chunks (+1 row halo each side)
    RB_GROUP = 4
    groups = []
    r = 0
    while r < NRB:
        groups.append(list(range(r, min(r + RB_GROUP, NRB))))
        r += RB_GROUP
    chunk_lo = [max(0, g[0] * RB - 1) for g in groups]
    chunk_hi = [min(H, (g[-1] + 1) * RB + 1) for g in groups]
    chunk_rows = max(hi - lo for lo, hi in zip(chunk_lo, chunk_hi))

    with tile.TileContext(nc) as tc, ExitStack() as ctx:
        wpool = ctx.enter_context(tc.tile_pool(name="wpool", bufs=1))
        xpool = ctx.enter_context(tc.tile_pool(name="xpool", bufs=8))
        opool = ctx.enter_context(tc.tile_pool(name="opool", bufs=4))
        ppool = ctx.enter_context(tc.tile_pool(name="ppool", bufs=8, space="PSUM"))

        wts_sb = {}
        for h in range(HALVES):
            wts_sb[h] = wpool.tile(
                [P, NTAPS, P], in_dt, name=f"wt{h}", tag=f"wt{h}"
            )
        # half-0 weights first: first matmul needs only them + chunk 0
        nc.sync.dma_start(out=wts_sb[0], in_=w_d[:, 0:NTAPS, :])
        if loop_iters > 1:
            nc.sync.dma_start(out=wts_sb[1], in_=w_d[:, NTAPS : 2 * NTAPS, :])

        def body():
            for u in range(B_PER * HALVES):
                img, h = divmod(u, HALVES)
                wt = wts_sb[h]
                chunks = []
                for c in range(len(groups)):
                    lo, hi = chunk_lo[c], chunk_hi[c]
                    xt = xpool.tile([P, chunk_rows, WP], in_dt,
                                    name=f"xt{u}_{c}", tag="xt")
                    nc.sync.dma_start(
                        out=xt[:, 0 : hi - lo, :],
                        in_=x_d[img, h * P : (h + 1) * P, lo:hi, :],
                    )
                    chunks.append(xt)
                if u == 0 and loop_iters == 1:
                    # half-1 weights queued behind unit 0's input chunks
                    nc.sync.dma_start(out=wts_sb[1], in_=w_d[:, NTAPS : 2 * NTAPS, :])
                for gi, grp in enumerate(groups):
                    lo = chunk_lo[gi]
                    xt = chunks[gi]
                    pss = {
                        rb: ppool.tile([P, RB, W], f32, name=f"ps{u}_{rb}", tag="ps")
                        for rb in grp
                    }
                    for i, (dh, dw) in enumerate(TAPS):
                        t = (dh + 1) * 3 + (dw + 1)
                        for rb in grp:
                            r0 = rb * RB
                            rs = max(r0, -dh)
                            re = min(r0 + RB, H - dh)
                            nc.tensor.matmul(
                                pss[rb][:, rs - r0 : re - r0, :],
                                wt[:, t, :],
                                xt[:, rs + dh - lo : re + dh - lo, dw + 1 : dw + 1 + W],
                                start=(i == 0),
                                stop=(i == NTAPS - 1),
                            )
                    for rb in grp:
                        r0 = rb * RB
                        ot = opool.tile([P, RB * W], f32)
                        nc.scalar.copy(ot, pss[rb].rearrange("p r c -> p (r c)"))
                        nc.sync.dma_start(
                            out=o_d[img, h * P : (h + 1) * P, r0 * W : r0 * W + RB * W],
                            in_=ot,
                        )

        if loop_iters > 1:
            with tc.For_i(0, loop_iters):
                body()
        else:
            body()
    nc._dedup_ldw = True
    nc.compile()
    return nc


# on-device conv input dtype: "fp16" measured ~14% faster than "f32r" with
# near-identical accuracy (see _build_program comment)
IN_DTYPE = "fp16"

_NP_DT = {"f32r": np.float32, "fp16": np.float16}


def _np_in_dtype():
    if IN_DTYPE == "bf16":
        import ml_dtypes

        return ml_dtypes.bfloat16
    return _NP_DT[IN_DTYPE]


def _get_program() -> bass.Bass:
    key = f"nc_{IN_DTYPE}"
    if key not in _prog_cache:
        _prog_cache[key] = _build_program(in_dtype=IN_DTYPE)
    return _prog_cache[key]


def _run(x: np.ndarray, w: np.ndarray, **run_kwargs):
    """Shard, run on 8 cores, gather. Returns (output, BassKernelResults)."""
    x = np.asarray(x, np.float32).reshape(B, C, H, W)
    xpad = np.zeros((B, C, H, WP), np.float32)
    xpad[:, :, :, 1 : 1 + W] = x
    wts = _build_weights(np.asarray(w, np.float32))
    if IN_DTYPE != "f32r":
        xpad = xpad.astype(_np_in_dtype())
        wts = wts.astype(_np_in_dtype())

    in_maps = [
        {"x": xpad[c * B_PER : (c + 1) * B_PER], "wts": wts.reshape(P, HALVES * NTAPS, P)}
        for c in range(N_CORES)
    ]
    nc = _get_program()
    res = run_bass_kernel_spmd(nc, in_maps, core_ids=list(range(N_CORES)), **run_kwargs)
    out = np.concatenate([res.results[c]["out"] for c in range(N_CORES)], axis=0)
    return out.reshape(B, C, H, W), res


def kernel(x: np.ndarray, w: np.ndarray) -> np.ndarray:
    out, _ = _run(x, w)
    return out



# revision 5
# speedup vs baseline: 1.2503x; 1.2503x over previous
"""Trainium2 kernel: depthwise (channel-multiplier-2) 3x3 conv + wing-swap + add.

Reference computes, for input x (B=32, C=256, H=W=56) and weights w (512,1,3,3):
    y[:, 2i], y[:, 2i+1] = conv3x3(x[:, i], w[2i]), conv3x3(x[:, i], w[2i+1])
    out[:, c] = y[:, 2c] + y[:, 2*swap(c)+1]
where swap() exchanges the two 4-channel wings inside each 8-channel butterfly.
Equivalently:  out[:, c] = conv3x3(x[:, c], w[2c]) + conv3x3(x[:, sc], w[2sc+1]),
sc = swap(c).

Strategy (8 NeuronCores, data-parallel over batch, 4 images/core):
  - channels on SBUF partitions, spatial pixels on the free dim; host pre-pads
    W by 1 col each side so every tap reads in-bounds
  - TWO taps per PE pass via concurrent 64x64 subarray tiles: the per-tap
    stationary matrix is block-diagonal (butterflies are 8 channels, so any
    aligned 64x64 diagonal block is closed under the wing swap).  Four weight
    tiles at tile_position (64a, 64A) run four matmuls CONCURRENTLY:
       row-group a = tap slot (two taps in flight), col-group A = 64-channel
       group.  5 generations of tap pairs (5+4 taps) replace 9 sequential
    full-array passes -> the PE streams 5/9 of the columns.
  - moving data for row-group a lives at partitions 64a..64a+63, so the same
    x bytes are DMA'd to both partition halves (2x input DMA, fp16)
  - PSUM bank(a, rb) accumulates across generations; evacuation: ACT copies
    the slot-1 bank to SBUF (DVE may read only one PSUM input, NCC_IBVF027),
    then DVE adds the slot-0 bank and writes fp16 (halves output DMA)
  - a post-compile pass (_dedup_ldweights) drops InstLdweights whose weights
    are already resident in exactly that 64x64 array rectangle, so each tap
    pair loads once per row-block group instead of once per matmul
  - measured ~82 us/core on HW (single-pass fp16 baseline ~107-115 us),
    absmax rel err ~6e-4 vs the fp32 reference
"""

import sys
from contextlib import ExitStack

import numpy as np

for _p in ("/opt/trn_rl_repo",):
    if _p not in sys.path:
        sys.path.insert(0, _p)

import concourse.bass as bass
import concourse.tile as tile
from concourse import bacc, mybir
from concourse.bass_utils import run_bass_kernel_spmd

B, C, H, W = 32, 256, 56, 56
WP = W + 2  # host-padded row width
N_CORES = 8
B_PER = B // N_CORES  # images per core
P = 128               # SBUF partitions
HALVES = C // P       # 2
RB = 8                # output rows per PSUM block
NRB = H // RB         # 7
BFLY = 8
WING = BFLY // 2
RB_GROUP = 4

# tap slots: each slot's FIRST tap has dh=0 so the gen-0 matmul writes the
# full row block (carries start=True for its PSUM bank)
SLOT_TAPS = {
    0: [(0, 0), (-1, -1), (-1, 0), (-1, 1), (1, 1)],
    1: [(0, -1), (0, 1), (1, -1), (1, 0)],
}
NGENS = 5
M64 = 64

_prog_cache = {}


def _swap_local64(m: np.ndarray) -> np.ndarray:
    b, r = m // BFLY, m % BFLY
    wng, pos = r // WING, r % WING
    return b * BFLY + (1 - wng) * WING + pos


def _build_weights(w: np.ndarray) -> np.ndarray:
    """wrep[128, HALVES*NGENS*2*M64] f32: row 64a+q holds, at flat index
    idx(h,gen,A), the 64-wide stationary column block of tap SLOT_TAPS[a][gen]
    mapping input ch (h*128+A*64+q) -> output ch (h*128+A*64+m)."""
    w2 = w.reshape(2 * C, 9).astype(np.float32)
    wrep = np.zeros((P, HALVES, NGENS, 2, M64), np.float32)
    q = np.arange(M64)
    sl = _swap_local64(q)
    for a in (0, 1):
        for gen, (dh, dw) in enumerate(SLOT_TAPS[a]):
            t = (dh + 1) * 3 + (dw + 1)
            for h in range(HALVES):
                for A in range(2):
                    g = h * P + A * M64 + q  # global input channel of row q
                    wrep[64 * a + q, h, gen, A, q] = w2[2 * g, t]
                    wrep[64 * a + q, h, gen, A, sl] = w2[2 * g + 1, t]
    return np.ascontiguousarray(wrep.reshape(P, HALVES * NGENS * 2 * M64))


def _rects_overlap(p1, s1, p2, s2):
    return (p1[0] < p2[0] + s2[0] and p2[0] < p1[0] + s1[0]
            and p1[1] < p2[1] + s2[1] and p2[1] < p1[1] + s1[1])


def _dedup_ldweights(m) -> int:
    """Drop InstLdweights whose weights are already resident in exactly that
    PE-array rectangle (per-subarray residency; a load clobbers overlapping
    rectangles).  Waits/updates/deps of a dropped load are merged into its
    InstMatmult.  Runs before generate_event_semaphores, which legalizes any
    multi-waits the merge produces."""
    ndrop = 0
    for fn in m.functions:
        for blk in fn.blocks:
            insts = blk.instructions
            resident = {}
            kept_name = {}
            pending = []
            remap = {}
            keep = []
            for inst in insts:
                tn = type(inst).__name__
                if inst.engine == mybir.EngineType.PE:
                    if tn == "InstLdweights":
                        tp = inst.tile_position
                        ts = inst.tile_size
                        pos = tuple(tp) if tp is not None else (0, 0)
                        size = tuple(ts) if ts is not None else (128, 128)
                        key = (
                            str(inst.ins),
                            str(inst.perf_mode),
                            str(inst.is_transpose),
                        )
                        rect = (pos, size)
                        if resident.get(rect) == key:
                            remap[inst.name] = kept_name[rect]
                            pending.append(inst)
                            ndrop += 1
                            continue
                        for r in [r for r in resident
                                  if r != rect and _rects_overlap(r[0], r[1],
                                                                  pos, size)]:
                            del resident[r]
                            kept_name.pop(r, None)
                        resident[rect] = key
                        kept_name[rect] = inst.name
                    elif tn == "InstMatmult":
                        for drop in pending:
                            si_d = drop.sync_info
                            if si_d is not None and (
                                len(si_d.on_wait) or len(si_d.on_update)
                            ):
                                si_m = inst.sync_info
                                ow = list(si_d.on_wait)
                                ou = list(si_d.on_update)
                                if si_m is not None:
                                    ow = list(si_m.on_wait) + ow
                                    ou = list(si_m.on_update) + ou
                                inst.sync_info = mybir.SyncInfo(
                                    on_wait=ow, on_update=ou
                                )
                            inst.merge_dependencies_from(drop)
                        pending = []
                    else:
                        resident.clear()
                        kept_name.clear()
                keep.append(inst)
            assert not pending, "dropped LDW with no following matmul"
            if len(keep) != len(insts):
                insts.clear()
                insts.extend(keep)
                for inst in insts:
                    inst.remap_dependency_names(remap)
    return ndrop


_orig_move_waits = bacc.Bacc.move_matmul_waits_to_ldweights


def _move_waits_and_dedup(self):
    _orig_move_waits(self)
    if getattr(self, "_dedup_ldw", False):
        self._ndedup = _dedup_ldweights(self.m)


bacc.Bacc.move_matmul_waits_to_ldweights = _move_waits_and_dedup


def _build_program(
    loop_iters: int = 1,
    timing_mode: bool = False,
    in_dtype: str = "fp16",
    out_fp16: bool = True,
) -> bass.Bass:
    # Bacc (not plain Bass): its compile() runs generate_event_semaphores,
    # which splits multi-wait instructions to satisfy the TRN2 1-wait limit
    nc = bacc.Bacc("TRN2", target_bir_lowering=False, debug=False)
    f32 = mybir.dt.float32
    in_dt = {"fp16": mybir.dt.float16, "bf16": mybir.dt.bfloat16}[in_dtype]
    w_dt = mybir.dt.float16
    out_dt = mybir.dt.float16 if out_fp16 else f32
    if timing_mode:
        # benchmark-only build: big tensors stay in device DRAM (garbage
        # contents) so wall-time isn't dominated by axon transfers
        x_d = nc.dram_tensor("x_int", [B_PER, C, H, WP], in_dt).ap()
        o_d = nc.dram_tensor("o_int", [B_PER, C, H * W], out_dt).ap()
        nc.dram_tensor("tiny", [1, 4], f32, kind="ExternalOutput")
    else:
        x_d = nc.dram_tensor("x", [B_PER, C, H, WP], in_dt, kind="ExternalInput").ap()
        o_d = nc.dram_tensor(
            "out", [B_PER, C, H * W], out_dt, kind="ExternalOutput"
        ).ap()
    w_d = nc.dram_tensor(
        "wts", [P, HALVES * NGENS * 2 * M64], w_dt, kind="ExternalInput"
    ).ap()

    groups = []
    r = 0
    while r < NRB:
        groups.append(list(range(r, min(r + RB_GROUP, NRB))))
        r += RB_GROUP
    chunk_lo = [max(0, g[0] * RB - 1) for g in groups]
    chunk_hi = [min(H, (g[-1] + 1) * RB + 1) for g in groups]
    chunk_rows = max(hi - lo for lo, hi in zip(chunk_lo, chunk_hi))

    last_gen = {0: NGENS - 1, 1: NGENS - 2}

    with tile.TileContext(nc) as tc, ExitStack() as ctx:
        wpool = ctx.enter_context(tc.tile_pool(name="wpool", bufs=1))
        xpool = ctx.enter_context(tc.tile_pool(name="xpool", bufs=6))
        opool = ctx.enter_context(tc.tile_pool(name="opool", bufs=4))
        ppool = ctx.enter_context(tc.tile_pool(name="ppool", bufs=8, space="PSUM"))

        wrep_sb = wpool.tile([P, HALVES * NGENS * 2, M64], w_dt, name="wrep")
        nc.sync.dma_start(out=wrep_sb, in_=w_d.rearrange("p (i m) -> p i m", m=M64))

        def body():
            for u in range(B_PER * HALVES):
                img, h = divmod(u, HALVES)
                chunks = []
                for c in range(len(groups)):
                    lo, hi = chunk_lo[c], chunk_hi[c]
                    xt = xpool.tile(
                        [P, 2, chunk_rows, WP], in_dt, name=f"xt{u}_{c}", tag="xt"
                    )
                    # same DRAM bytes land at both tap-slot partition groups
                    for a in (0, 1):
                        for A in (0, 1):
                            nc.sync.dma_start(
                                out=xt[64 * a : 64 * (a + 1), A, 0 : hi - lo, :],
                                in_=x_d[
                                    img,
                                    h * P + A * M64 : h * P + (A + 1) * M64,
                                    lo:hi,
                                    :,
                                ],
                            )
                    chunks.append(xt)
                for gi, grp in enumerate(groups):
                    lo = chunk_lo[gi]
                    xt = chunks[gi]
                    pss = {
                        (a, rb): ppool.tile(
                            [P, RB, W], f32, name=f"ps{u}_{a}_{rb}", tag="ps"
                        )
                        for a in (0, 1)
                        for rb in grp
                    }
                    for gen in range(NGENS):
                        for rb in grp:
                            r0 = rb * RB
                            for A in (0, 1):
                                for a in (0, 1):
                                    if gen > last_gen[a]:
                                        continue
                                    dh, dw = SLOT_TAPS[a][gen]
                                    rs = max(r0, -dh)
                                    re = min(r0 + RB, H - dh)
                                    idx = (h * NGENS + gen) * 2 + A
                                    nc.tensor.matmul(
                                        pss[(a, rb)][
                                            64 * A : 64 * (A + 1),
                                            rs - r0 : re - r0,
                                            :,
                                        ],
                                        wrep_sb[64 * a : 64 * (a + 1), idx, :],
                                        xt[
                                            64 * a : 64 * (a + 1),
                                            A,
                                            rs + dh - lo : re + dh - lo,
                                            dw + 1 : dw + 1 + W,
                                        ],
                                        start=(gen == 0),
                                        stop=(gen == last_gen[a]),
                                        tile_position=(64 * a, 64 * A),
                                    )
                    ot = opool.tile(
                        [P, len(grp) * RB * W], out_dt, name=f"ot{u}_{gi}", tag="ot"
                    )
                    for j, rb in enumerate(grp):
                        # DVE may read only ONE input from PSUM (NCC_IBVF027):
                        # ACT stages the slot-1 bank to SBUF, DVE adds the
                        # slot-0 bank and converts to the output dtype.
                        tmp = opool.tile(
                            [P, RB * W], f32, name=f"tp{u}_{gi}_{j}", tag="tp"
                        )
                        nc.scalar.copy(
                            tmp, pss[(1, rb)].rearrange("p r c -> p (r c)")
                        )
                        nc.vector.scalar_tensor_tensor(
                            out=ot[:, j * RB * W : (j + 1) * RB * W],
                            in0=pss[(0, rb)].rearrange("p r c -> p (r c)"),
                            scalar=1.0,
                            in1=tmp,
                            op0=mybir.AluOpType.mult,
                            op1=mybir.AluOpType.add,
                        )
                    r0 = grp[0] * RB
                    nc.sync.dma_start(
                        out=o_d[
                            img,
                            h * P : (h + 1) * P,
                            r0 * W : r0 * W + len(grp) * RB * W,
                        ],
                        in_=ot,
                    )

        if loop_iters > 1:
            with tc.For_i(0, loop_iters):
                body()
        else:
            body()
    nc._dedup_ldw = True
    nc.compile()
    return nc


IN_DTYPE = "fp16"
OUT_FP16 = True


def _np_in_dtype():
    return np.float16


def _get_program() -> bass.Bass:
    key = (IN_DTYPE, OUT_FP16)
    if key not in _prog_cache:
        _prog_cache[key] = _build_program(in_dtype=IN_DTYPE, out_fp16=OUT_FP16)
    return _prog_cache[key]


def _timing_in_maps(w: np.ndarray) -> list:
    wts = _build_weights(np.asarray(w, np.float32)).astype(np.float16)
    return [{"wts": wts} for _ in range(N_CORES)]


def _run(x: np.ndarray, w: np.ndarray, **run_kwargs):
    """Shard, run on 8 cores, gather. Returns (output, BassKernelResults)."""
    x = np.asarray(x, np.float32).reshape(B, C, H, W)
    xpad = np.zeros((B, C, H, WP), np.float32)
    xpad[:, :, :, 1 : 1 + W] = x
    xpad = xpad.astype(_np_in_dtype())
    wts = _build_weights(np.asarray(w, np.float32)).astype(np.float16)

    in_maps = [
        {"x": xpad[c * B_PER : (c + 1) * B_PER], "wts": wts}
        for c in range(N_CORES)
    ]
    nc = _get_program()
    res = run_bass_kernel_spmd(nc, in_maps, core_ids=list(range(N_CORES)), **run_kwargs)
    out = np.concatenate([res.results[c]["out"] for c in range(N_CORES)], axis=0)
    return out.astype(np.float32).reshape(B, C, H, W), res


def kernel(x: np.ndarray, w: np.ndarray) -> np.ndarray:
    out, _ = _run(x, w)
    return out
